# revision 17
# baseline (speedup 1.0000x reference)
"""Trainium2 Bass kernel for a 2-layer bipartite heterogeneous GraphSAGE.

Contract: kernel(**inputs) takes FULL unsharded numpy inputs and returns the
FULL output ([8] float32 softmax vector). Internally shards across 8
NeuronCores via bass_utils.run_bass_kernel_spmd.

Strategy (memory-bound gather/segment-sum workload):
  - Destination-node sharding: nodes of each side are relabeled host-side in
    degree-sorted, core-interleaved order so that (a) each core owns a
    contiguous range of 49 tiles x 128 dst nodes, (b) per-128-tile degrees are
    nearly uniform (minimal gather padding), (c) per-core total edge counts are
    balanced, and (d) an AllGather of per-core row shards directly rebuilds the
    full relabeled feature table.
  - Segment-mean: host sorts edges by (new) dst and emits per-core [128, S]
    int32 gather-index matrices (column blocks per dst tile, sentinel index ->
    an all-zero row). On device: batched indirect-DMA gathers (<=16 edge slots
    per instruction) accumulated with the DMA CCE (cce_op=add), then a short
    in-SBUF tree reduction, then a per-partition 1/deg scale.
  - Linear parts: per dst tile, PE-transpose agg and use PSUM accumulation:
    psum = aggT.T@Wl + rootT.T@Wr + ones.T@bias, ReLU on the scalar engine.
  - Layer-1 sources come from AllGathered layer-0 tables; layer-1 root terms
    come from core-local transposed copies written during layer 0.
  - Readout: per-tile relu(h2@Wn+bn) masked and accumulated in SBUF; partition
    reduction via matmul with a ones vector; tiny AllReduce; fc + softmax on
    device (Wfc pre-scaled by 1/N host-side to fold the mean).
"""

import os
import sys

import numpy as np

if "/opt/trn_rl_repo" not in sys.path:
    sys.path.insert(0, "/opt/trn_rl_repo")

os.environ.setdefault("MYCRO_LOCAL_CACHE", "1")

# ---------------------------------------------------------------- config

NCORES = 8
P = 128          # partitions / dst-tile size
GMAX = 64        # max gather slots (edges per dst) per indirect DMA


class Cfg:
    def __init__(self, N=50000, E=1600000, TPC=49, FA=128, FB=64, HID=128, OUT=8):
        self.N, self.E, self.TPC = N, E, TPC
        self.FA, self.FB, self.HID, self.OUT = FA, FB, HID, OUT
        self.CP = TPC * P               # rows per core
        self.NP = NCORES * self.CP      # padded node count
        self.NSRC = self.NP             # source table rows
        self.SENT = self.NP - 1         # sentinel row: last dummy row, always 0
        assert self.NP > N              # need at least one dummy row (sentinel)


REAL = Cfg()

# ---------------------------------------------------------------- host prep


def _relabel(deg, cfg):
    """Degree-desc, core-interleaved relabeling.

    Returns (newid_of_old [N], dummy_newids) where core c owns new ids
    [c*CP, (c+1)*CP) and global degree-rank q sits at tile t=q//P, partition
    p=q%P, mapped to core c=t%NCORES, local tile j=t//NCORES.
    """
    N, NP = cfg.N, cfg.NP
    order = np.argsort(-deg, kind="stable")
    q = np.arange(NP)
    t, p = q // P, q % P
    c, j = t % NCORES, t // NCORES
    newpos = c * cfg.CP + j * P + p
    newid_of_old = np.empty(N, np.int64)
    newid_of_old[order] = newpos[:N]
    return newid_of_old, newpos[N:]


def _build_rel(edge, dst_newid, src_newid, cfg):
    """Per-core gather index arrays + per-core inv-degree + shared schedule.

    Returns (idx [NCORES,128,S] int32, icnt [NCORES,128,TPC] f32, Kj [TPC]).
    """
    N, E, TPC, CP, NP = cfg.N, edge.shape[1], cfg.TPC, cfg.CP, cfg.NP
    dst_new = dst_newid[edge[1]]
    src_new = src_newid[edge[0]]
    deg_new = np.bincount(dst_new, minlength=NP)

    deg_r = deg_new.reshape(NCORES, TPC, P)
    Kj = deg_r.max(axis=(0, 2)).astype(np.int64)          # shared schedule
    col0 = np.concatenate([[0], np.cumsum(Kj)]).astype(np.int64)
    S = int(col0[-1])

    ordr = np.argsort(dst_new, kind="stable")
    ds, ss = dst_new[ordr], src_new[ordr]
    starts = np.searchsorted(ds, np.arange(NP))
    rank = np.arange(E, dtype=np.int64) - starts[ds]
    cc = ds // CP
    loc = ds % CP
    jj, pp = loc // P, loc % P
    cols = col0[jj] + rank

    idx = np.full((NCORES, P, S), cfg.SENT, np.int32)
    idx[cc, pp, cols] = ss.astype(np.int32)

    icnt = (1.0 / np.maximum(deg_r, 1)).astype(np.float32)  # [C, TPC, P]
    icnt = np.ascontiguousarray(icnt.transpose(0, 2, 1))    # [C, P, TPC]
    return idx, icnt, Kj


def preprocess(x_a, x_b, edge_ab, edge_ba, params, cfg):
    """All numpy. Returns (in_maps list of dicts, Kj_ba, Kj_ab)."""
    N = cfg.N
    f32 = np.float32
    x_a = np.asarray(x_a, f32)
    x_b = np.asarray(x_b, f32)
    edge_ab = np.asarray(edge_ab)
    edge_ba = np.asarray(edge_ba)

    deg_a = np.bincount(np.asarray(edge_ba[1], np.int64), minlength=N)
    deg_b = np.bincount(np.asarray(edge_ab[1], np.int64), minlength=N)
    newid_a, dummy_a = _relabel(deg_a, cfg)
    newid_b, dummy_b = _relabel(deg_b, cfg)

    idx_ba, icnt_a, Kj_ba = _build_rel(edge_ba, newid_a, newid_b, cfg)
    idx_ab, icnt_b, Kj_ab = _build_rel(edge_ab, newid_b, newid_a, cfg)

    xa_src = np.zeros((cfg.NSRC, cfg.FA), f32)
    xa_src[newid_a] = x_a
    xb_src = np.zeros((cfg.NSRC, cfg.FB), f32)
    xb_src[newid_b] = x_b

    mask = np.ones((NCORES, cfg.TPC, P), f32)
    for dm in (dummy_a,):  # dummy positions identical for both relabelings
        c = dm // cfg.CP
        loc = dm % cfg.CP
        mask[c, loc // P, loc % P] = 0.0
    mask = np.ascontiguousarray(mask.transpose(0, 2, 1))  # [C, P, TPC]

    def W(a):
        return np.ascontiguousarray(np.asarray(a, f32))

    def B(a):
        return np.ascontiguousarray(np.asarray(a, f32).reshape(1, -1))

    p0ba, p0ab = params["layer0"]["ba"], params["layer0"]["ab"]
    p1ba, p1ab = params["layer1"]["ba"], params["layer1"]["ab"]
    wfc = np.asarray(params["Wfc"], f32) / np.float32(N)

    rep = {
        "xa_src": xa_src, "xb_src": xb_src,
        "w0_ba_l": W(p0ba["Wl"]), "w0_ba_r": W(p0ba["Wr"]), "b0_ba": B(p0ba["bl"]),
        "w0_ab_l": W(p0ab["Wl"]), "w0_ab_r": W(p0ab["Wr"]), "b0_ab": B(p0ab["bl"]),
        "w1_ba_l": W(p1ba["Wl"]), "w1_ba_r": W(p1ba["Wr"]), "b1_ba": B(p1ba["bl"]),
        "w1_ab_l": W(p1ab["Wl"]), "w1_ab_r": W(p1ab["Wr"]), "b1_ab": B(p1ab["bl"]),
        "wn": W(params["Wn"]), "bn": B(params["bn"]),
        "wfca": W(wfc[: cfg.HID]), "wfcb": W(wfc[cfg.HID:]), "bfc": B(params["bfc"]),
        "ones_row": np.ones((1, P), f32),
        "ones_col": np.ones((P, 1), f32),
        "ident": np.eye(P, dtype=f32),
    }

    in_maps = []
    for c in range(NCORES):
        sl = slice(c * cfg.CP, (c + 1) * cfg.CP)
        m = dict(rep)
        m["xaT"] = np.ascontiguousarray(xa_src[sl].T)
        m["xbT"] = np.ascontiguousarray(xb_src[sl].T)
        m["idx_ba"] = idx_ba[c]
        m["idx_ab"] = idx_ab[c]
        m["icnt_a"] = icnt_a[c]
        m["icnt_b"] = icnt_b[c]
        m["mask"] = mask[c]
        in_maps.append(m)
    return in_maps, Kj_ba, Kj_ab


# ---------------------------------------------------------------- device


def build_program(cfg, Kj_ba, Kj_ab):
    import concourse.bass as bass
    import concourse.bacc as bacc
    import concourse.mybir as mybir
    import concourse.tile as tile
    from contextlib import ExitStack

    f32 = mybir.dt.float32
    i32 = mybir.dt.int32
    ADD = mybir.AluOpType.add
    BYP = mybir.AluOpType.bypass
    Relu = mybir.ActivationFunctionType.Relu
    Exp = mybir.ActivationFunctionType.Exp
    AX = mybir.AxisListType.X

    TPC, CP, NP, NSRC = cfg.TPC, cfg.CP, cfg.NP, cfg.NSRC
    FA, FB, HID, OUT = cfg.FA, cfg.FB, cfg.HID, cfg.OUT

    S_ba = int(np.sum(Kj_ba))
    S_ab = int(np.sum(Kj_ab))
    col0_ba = np.concatenate([[0], np.cumsum(Kj_ba)]).astype(int)
    col0_ab = np.concatenate([[0], np.cumsum(Kj_ab)]).astype(int)

    nc = bacc.Bacc(None, num_devices=NCORES)

    def din(name, shape, dt=f32):
        return nc.dram_tensor(name, list(shape), dt, kind="ExternalInput")

    xa_src_d = din("xa_src", [NSRC, FA])
    xb_src_d = din("xb_src", [NSRC, FB])
    xaT_d = din("xaT", [FA, CP])
    xbT_d = din("xbT", [FB, CP])
    idx_ba_d = din("idx_ba", [P, S_ba], i32)
    idx_ab_d = din("idx_ab", [P, S_ab], i32)
    icnt_a_d = din("icnt_a", [P, TPC])
    icnt_b_d = din("icnt_b", [P, TPC])
    mask_d = din("mask", [P, TPC])
    w0_ba_l_d = din("w0_ba_l", [FB, HID]); w0_ba_r_d = din("w0_ba_r", [FA, HID])
    w0_ab_l_d = din("w0_ab_l", [FA, HID]); w0_ab_r_d = din("w0_ab_r", [FB, HID])
    w1_ba_l_d = din("w1_ba_l", [HID, HID]); w1_ba_r_d = din("w1_ba_r", [HID, HID])
    w1_ab_l_d = din("w1_ab_l", [HID, HID]); w1_ab_r_d = din("w1_ab_r", [HID, HID])
    b0_ba_d = din("b0_ba", [1, HID]); b0_ab_d = din("b0_ab", [1, HID])
    b1_ba_d = din("b1_ba", [1, HID]); b1_ab_d = din("b1_ab", [1, HID])
    wn_d = din("wn", [HID, HID]); bn_d = din("bn", [1, HID])
    wfca_d = din("wfca", [HID, OUT]); wfcb_d = din("wfcb", [HID, OUT])
    bfc_d = din("bfc", [1, OUT])
    ones_row_d = din("ones_row", [1, P])
    ones_col_d = din("ones_col", [P, 1])
    ident_d = din("ident", [P, P])

    out_ext = nc.dram_tensor("out", [1, OUT], f32, kind="ExternalOutput")

    rg = [list(range(NCORES))]

    with ExitStack() as ctx:
        tc = ctx.enter_context(tile.TileContext(nc))
        sb = ctx.enter_context(tc.tile_pool(name="persist", bufs=1))
        dram = ctx.enter_context(tc.tile_pool(name="dram", bufs=1, space="DRAM"))
        accp = ctx.enter_context(tc.tile_pool(name="accp", bufs=2))
        sp = ctx.enter_context(tc.tile_pool(name="small", bufs=3))
        pst = ctx.enter_context(tc.tile_pool(name="pst", bufs=2, space="PSUM"))
        pso = ctx.enter_context(tc.tile_pool(name="pso", bufs=2, space="PSUM"))
        pst2 = ctx.enter_context(tc.tile_pool(name="pst2", bufs=2, space="PSUM"))
        psr = ctx.enter_context(tc.tile_pool(name="psr", bufs=2, space="PSUM"))

        def load(dten, shape, dt=f32, name=None):
            t = sb.tile(list(shape), dt, name=name or (dten.name + "_s"))
            nc.sync.dma_start(t[:], dten[:])
            return t

        idx_ba_s = load(idx_ba_d, [P, S_ba], i32)
        idx_ab_s = load(idx_ab_d, [P, S_ab], i32)
        icnt_a_s = load(icnt_a_d, [P, TPC])
        icnt_b_s = load(icnt_b_d, [P, TPC])
        mask_s = load(mask_d, [P, TPC])
        w0_ba_l_s = load(w0_ba_l_d, [FB, HID]); w0_ba_r_s = load(w0_ba_r_d, [FA, HID])
        w0_ab_l_s = load(w0_ab_l_d, [FA, HID]); w0_ab_r_s = load(w0_ab_r_d, [FB, HID])
        w1_ba_l_s = load(w1_ba_l_d, [HID, HID]); w1_ba_r_s = load(w1_ba_r_d, [HID, HID])
        w1_ab_l_s = load(w1_ab_l_d, [HID, HID]); w1_ab_r_s = load(w1_ab_r_d, [HID, HID])
        b0_ba_s = load(b0_ba_d, [1, HID]); b0_ab_s = load(b0_ab_d, [1, HID])
        b1_ba_s = load(b1_ba_d, [1, HID]); b1_ab_s = load(b1_ab_d, [1, HID])
        wn_s = load(wn_d, [HID, HID]); bn_s = load(bn_d, [1, HID])
        wfca_s = load(wfca_d, [HID, OUT]); wfcb_s = load(wfcb_d, [HID, OUT])
        bfc_s = load(bfc_d, [1, OUT])
        ones_row_s = load(ones_row_d, [1, P])
        ones_col_s = load(ones_col_d, [P, 1])
        ident_s = load(ident_d, [P, P])

        rsum_a = sb.tile([P, HID], f32, name="rsum_a")
        rsum_b = sb.tile([P, HID], f32, name="rsum_b")
        nc.vector.memset(rsum_a[:], 0.0)
        nc.vector.memset(rsum_b[:], 0.0)

        h1a_shard = dram.tile([CP, HID], f32, name="h1a_shard")
        h1b_shard = dram.tile([CP, HID], f32, name="h1b_shard")
        h1aT_loc = dram.tile([HID, CP], f32, name="h1aT_loc")
        h1bT_loc = dram.tile([HID, CP], f32, name="h1bT_loc")
        h1a_full = dram.tile([NSRC, HID], f32, name="h1a_full", addr_space="Shared")
        h1b_full = dram.tile([NSRC, HID], f32, name="h1b_full", addr_space="Shared")
        pr_in = dram.tile([P, 2], f32, name="pr_in")
        pr_out = dram.tile([P, 2], f32, name="pr_out", addr_space="Shared")

        def gather_phase(idx_s, col0, Kj, src_ap, F, icnt_s, rootT_ap, rootF,
                         wl_s, wr_s, bias_s, sink, tag):
            for j in range(TPC):
                K = int(Kj[j])
                if K == 0:
                    agg = accp.tile([P, F], f32, tag="agg", name=f"agg_{tag}_{j}")
                    nc.vector.memset(agg[:], 0.0)
                else:
                    # disjoint chunk regions, one indirect DMA each; local
                    # tree-reduce per region (waits on exactly one DMA), then
                    # combine region heads on the DVE (same-engine FIFO).
                    agg = accp.tile([P, K * F], f32, tag="agg",
                                    name=f"agg_{tag}_{j}")
                    bases = []
                    off = 0
                    while off < K:
                        w = min(GMAX, K - off)
                        nc.gpsimd.indirect_dma_start(
                            out=agg[:, off * F: (off + w) * F],
                            out_offset=None,
                            in_=src_ap,
                            in_offset=bass.IndirectOffsetOnAxis(
                                ap=idx_s[:, col0[j] + off: col0[j] + off + w],
                                axis=0,
                            ),
                        )
                        bases.append((off, w))
                        off += w
                    for b, w in bases:
                        while w > 1:
                            h = w // 2
                            nc.vector.tensor_tensor(
                                out=agg[:, b * F: (b + h) * F],
                                in0=agg[:, b * F: (b + h) * F],
                                in1=agg[:, (b + w - h) * F: (b + w) * F],
                                op=ADD,
                            )
                            w -= h
                    for b, _ in bases[1:]:
                        nc.vector.tensor_tensor(
                            out=agg[:, :F],
                            in0=agg[:, :F],
                            in1=agg[:, b * F: (b + 1) * F],
                            op=ADD,
                        )
                nc.vector.tensor_scalar_mul(agg[:, :F], agg[:, :F],
                                            icnt_s[:, j: j + 1])
                pt = pst.tile([F, P], f32, tag="pt", name=f"pt_{tag}_{j}")
                nc.tensor.transpose(pt[:], agg[:, :F], ident_s[:])
                aggT = sp.tile([F, P], f32, tag="aggT", name=f"aggT_{tag}_{j}")
                nc.vector.tensor_copy(aggT[:], pt[:])
                rootT = sp.tile([rootF, P], f32, tag="rootT",
                                name=f"rootT_{tag}_{j}")
                nc.sync.dma_start(rootT[:], rootT_ap[:, j * P: (j + 1) * P])
                po = pso.tile([P, HID], f32, tag="po", name=f"po_{tag}_{j}")
                nc.tensor.matmul(po[:], lhsT=aggT[:], rhs=wl_s[:],
                                 start=True, stop=False)
                nc.tensor.matmul(po[:], lhsT=rootT[:], rhs=wr_s[:],
                                 start=False, stop=False)
                nc.tensor.matmul(po[:], lhsT=ones_row_s[:], rhs=bias_s[:],
                                 start=False, stop=True)
                hs = sp.tile([P, HID], f32, tag="hs", name=f"hs_{tag}_{j}")
                nc.scalar.activation(hs[:], po[:], Relu)
                sink(j, hs)

        def sink_l0(shard, hT_loc, tag):
            def s(j, hs):
                # zero dummy rows so the gathered table's sentinel row is 0
                nc.vector.tensor_scalar_mul(hs[:], hs[:], mask_s[:, j: j + 1])
                nc.sync.dma_start(shard[j * P: (j + 1) * P, :], hs[:])
                pt2 = pst2.tile([HID, P], f32, tag="pt2", name=f"pt2_{tag}_{j}")
                nc.tensor.transpose(pt2[:], hs[:], ident_s[:])
                hT = sp.tile([HID, P], f32, tag="hT", name=f"hT_{tag}_{j}")
                nc.vector.tensor_copy(hT[:], pt2[:])
                nc.sync.dma_start(hT_loc[:, j * P: (j + 1) * P], hT[:])
            return s

        def sink_l1(rsum, tag):
            def s(j, hs):
                pt2 = pst2.tile([HID, P], f32, tag="pt2", name=f"pt2_{tag}_{j}")
                nc.tensor.transpose(pt2[:], hs[:], ident_s[:])
                h2T = sp.tile([HID, P], f32, tag="hT", name=f"h2T_{tag}_{j}")
                nc.vector.tensor_copy(h2T[:], pt2[:])
                pr = psr.tile([P, HID], f32, tag="pr", name=f"pr_{tag}_{j}")
                nc.tensor.matmul(pr[:], lhsT=h2T[:], rhs=wn_s[:],
                                 start=True, stop=False)
                nc.tensor.matmul(pr[:], lhsT=ones_row_s[:], rhs=bn_s[:],
                                 start=False, stop=True)
                rt = sp.tile([P, HID], f32, tag="rt", name=f"rt_{tag}_{j}")
                nc.scalar.activation(rt[:], pr[:], Relu)
                nc.vector.tensor_scalar_mul(rt[:], rt[:], mask_s[:, j: j + 1])
                nc.vector.tensor_tensor(out=rsum[:], in0=rsum[:], in1=rt[:],
                                        op=ADD)
            return s

        # ---- layer 0, dst=A (relation ba: sources in B)
        gather_phase(idx_ba_s, col0_ba, Kj_ba, xb_src_d[:, :], FB, icnt_a_s,
                     xaT_d, FA, w0_ba_l_s, w0_ba_r_s, b0_ba_s,
                     sink_l0(h1a_shard, h1aT_loc, "l0a"), "l0a")
        nc.gpsimd.collective_compute(
            "AllGather", BYP, replica_groups=rg,
            ins=[h1a_shard[:, :].opt()], outs=[h1a_full[:, :].opt()])

        # ---- layer 0, dst=B (relation ab: sources in A)
        gather_phase(idx_ab_s, col0_ab, Kj_ab, xa_src_d[:, :], FA, icnt_b_s,
                     xbT_d, FB, w0_ab_l_s, w0_ab_r_s, b0_ab_s,
                     sink_l0(h1b_shard, h1bT_loc, "l0b"), "l0b")
        nc.gpsimd.collective_compute(
            "AllGather", BYP, replica_groups=rg,
            ins=[h1b_shard[:, :].opt()], outs=[h1b_full[:, :].opt()])

        # ---- layer 1, dst=B (sources = h1a_full), readout into rsum_b
        gather_phase(idx_ab_s, col0_ab, Kj_ab, h1a_full[:, :], HID, icnt_b_s,
                     h1bT_loc, HID, w1_ab_l_s, w1_ab_r_s, b1_ab_s,
                     sink_l1(rsum_b, "l1b"), "l1b")

        # ---- layer 1, dst=A (sources = h1b_full), readout into rsum_a
        gather_phase(idx_ba_s, col0_ba, Kj_ba, h1b_full[:, :], HID, icnt_a_s,
                     h1aT_loc, HID, w1_ba_l_s, w1_ba_r_s, b1_ba_s,
                     sink_l1(rsum_a, "l1a"), "l1a")

        # ---- readout: partition-reduce, AllReduce, fc, softmax
        pfa = psr.tile([HID, 1], f32, tag="pr", name="pfa")
        nc.tensor.matmul(pfa[:], lhsT=rsum_a[:], rhs=ones_col_s[:],
                         start=True, stop=True)
        pfb = psr.tile([HID, 1], f32, tag="pr", name="pfb")
        nc.tensor.matmul(pfb[:], lhsT=rsum_b[:], rhs=ones_col_s[:],
                         start=True, stop=True)
        pack = sb.tile([P, 2], f32, name="pack")
        nc.vector.tensor_copy(pack[:, 0:1], pfa[:])
        nc.vector.tensor_copy(pack[:, 1:2], pfb[:])
        nc.sync.dma_start(pr_in[:, :], pack[:])
        nc.gpsimd.collective_compute(
            "AllReduce", ADD, replica_groups=rg,
            ins=[pr_in[:, :].opt()], outs=[pr_out[:, :].opt()])
        red = sb.tile([P, 2], f32, name="red")
        nc.sync.dma_start(red[:], pr_out[:, :])

        pv = pso.tile([1, OUT], f32, tag="po", name="pv")
        nc.tensor.matmul(pv[:], lhsT=red[:, 0:1], rhs=wfca_s[:],
                         start=True, stop=False)
        nc.tensor.matmul(pv[:], lhsT=red[:, 1:2], rhs=wfcb_s[:],
                         start=False, stop=False)
        nc.tensor.matmul(pv[:], lhsT=ones_row_s[:, 0:1], rhs=bfc_s[:],
                         start=False, stop=True)

        mx = sb.tile([1, 1], f32, name="mx")
        nc.vector.reduce_max(mx[:], pv[:], axis=AX)
        vs = sb.tile([1, OUT], f32, name="vs")
        nc.vector.tensor_scalar_sub(vs[:], pv[:], mx[:])
        es = sb.tile([1, OUT], f32, name="es")
        nc.scalar.activation(es[:], vs[:], Exp)
        sm = sb.tile([1, 1], f32, name="sm")
        nc.vector.reduce_sum(sm[:], es[:], axis=AX)
        rc = sb.tile([1, 1], f32, name="rc")
        nc.vector.reciprocal(rc[:], sm[:])
        osf = sb.tile([1, OUT], f32, name="osf")
        nc.vector.tensor_scalar_mul(osf[:], es[:], rc[:])
        nc.sync.dma_start(out_ext[:, :], osf[:])

    if not nc.is_finalized():
        nc.finalize()
    return nc


# ---------------------------------------------------------------- entry


def run(inputs, cfg=REAL, trace=False, trace_kwargs=None):
    from concourse.bass_utils import run_bass_kernel_spmd

    in_maps, Kj_ba, Kj_ab = preprocess(
        inputs["x_a"], inputs["x_b"], inputs["edge_ab"], inputs["edge_ba"],
        inputs["params"], cfg)
    nc = build_program(cfg, Kj_ba, Kj_ab)
    res = run_bass_kernel_spmd(
        nc, in_maps, list(range(NCORES)), trace=trace,
        **(trace_kwargs or {}))
    out = np.asarray(res.results[0]["out"], np.float32).reshape(cfg.OUT)
    return out, res


def kernel(**inputs) -> np.ndarray:
    out, _ = run(inputs, REAL, trace=False)
    return out
